# revision 3
# baseline (speedup 1.0000x reference)
"""MoE layer (top-2 routing, E=8 experts) on 8 Trainium2 NeuronCores.

Strategy (expert parallelism):
  - Host: gate (float64) -> top-2 routing; per-expert token gather.
  - Device core e: the expert-e MLP over a capacity of C=1024 tokens
    (the mean load), computed with fp8 DoubleRow matmuls (2 contraction
    rows per cycle, ~4x the bf16 PE rate measured on hw) plus error
    compensation to stay inside the 2e-2 gate:
      * Stage 1 (x @ W1): fully hi/lo-compensated fp8. Host splits both
        operands X = xH + xL, W = wH + wL into fp8 pairs; the device
        accumulates xH@wH + xL@wH + xH@wL in PSUM (3 DoubleRow matmuls
        per k-slab pair), giving ~bf16-level precision.
      * Stage 2 (h @ W2): dither-pair fp8. The PSUM h is evicted twice
        (ScalarE relu+scale) as hA = f8(SH*h) and hB = f8(SH*h*DSHIFT)
        on phase-shifted fp8 grids; the host prepares W2 pairs (w2a,
        w2b) where w2b greedily cancels w2a's rounding error. Each
        slab is then ONE DoubleRow matmul hA@w2a + hB@w2b whose noise
        is ~half of naive fp8 per source.
    Measured end-to-end rel err ~1.6e-2 (gate: 2e-2); naive fp8 would
    be 4.3e-2 and pure bf16 2.9e-3 but ~1.4x slower.
  - Tokens over capacity (expert load > 1024, ~1-3% of token-expert
    pairs) are computed exactly on host in float64 during the combine
    (standard capacity-factor expert parallelism, with exact fallback
    instead of token dropping).
  - Chunks are 2x512 columns: 512 is the ISA max matmul output width,
    and per-instruction overhead (~140ns: stationary-load + issue)
    dominates narrower chunks, so minimizing instruction count wins.
  - Timing loop (repeats>1): tc.For_i puts an all-engine barrier at the
    back-edge which would expose the x load; the body is ROTATED
    [load x(k); stage2(k-1); stage1(k)] so stage-2 (whose inputs are
    already SBUF-resident) restarts the PE immediately while x streams.
"""
import sys

sys.path.insert(0, "/opt/trn_rl_repo")

import numpy as np
import ml_dtypes

N, D, H, E, TOP_K = 4096, 1024, 2048, 8, 2
C = 1024            # per-expert device capacity (= mean load)
CTS = (512, 512)    # psum chunk widths (512 = ISA max matmul out)
COFF = (0, 512)
NCT = len(CTS)
DT = D // 128       # 8
HT = H // 128       # 16

FP8 = ml_dtypes.float8_e4m3
SX, SW1, SH, SW2 = 32.0, 1024.0, 32.0, 1024.0   # fp8 pre-scales
DSHIFT = 1.0 + 2.0 ** -4                        # stage-2 dither grid shift

_CACHE = {}


def _build_bass(repeats=1):
    import concourse.tile as tile
    from concourse import bacc, mybir

    f32 = mybir.dt.float32
    f8 = mybir.dt.float8e4
    DR = mybir.MatmulPerfMode.DoubleRow

    nc = bacc.Bacc("TRN2", target_bir_lowering=False, debug=False, num_devices=E)

    # Host-prearranged SBUF layouts, partition dim first:
    #   xt: [128, 2(ver) * DT * C]   ver 0 = xH, 1 = xL  (fp8, scaled SX)
    #   w1: [128, 2(ver) * DT * H]   ver 0 = wH, 1 = wL  (fp8, scaled 2*SW1)
    #   w2: [128, HT * 2(ver) * D]   ver 0 = w2a, 1 = w2b (fp8, scaled SW2)
    #   b1t:[128, 2 * HT]            col 2h+v = bias for version v of h-tile
    xt = nc.dram_tensor("xt", [128, 2 * DT * C], f8, kind="ExternalInput").ap()
    w1 = nc.dram_tensor("w1", [128, 2 * DT * H], f8, kind="ExternalInput").ap()
    w2 = nc.dram_tensor("w2", [128, HT * 2 * D], f8, kind="ExternalInput").ap()
    b1t = nc.dram_tensor("b1t", [128, 2 * HT], f32, kind="ExternalInput").ap()
    yt = nc.dram_tensor("yt", [D, C], f32, kind="ExternalOutput").ap()

    relu = mybir.ActivationFunctionType.Relu
    sA = SH / (2 * SX * SW1)      # psum -> SH*h eviction scale
    sB = sA * DSHIFT

    with tile.TileContext(nc) as tc:
        with (
            tc.tile_pool(name="persist", bufs=1) as persist,
            tc.tile_pool(name="psum", bufs=2, space="PSUM") as psum,
        ):
            b1sb = persist.tile([128, 2 * HT], f32, name="b1sb", tag="b1")
            w1sb = persist.tile([128, 2, DT, H], f8, name="w1sb", tag="w1")
            xtsb = persist.tile([128, 2, DT, C], f8, name="xtsb", tag="xt")
            w2sb = persist.tile([128, HT, 2, D], f8, name="w2sb", tag="w2")
            ht = persist.tile([128, HT, 2, C], f8, name="ht", tag="ht")

            def emit_weight_loads():
                # Static weights: loaded once, SBUF-resident across calls.
                nc.sync.dma_start(b1sb[:], b1t[:])
                for v in range(2):
                    for d in range(DT):
                        nc.sync.dma_start(
                            w1sb[:, v, d, :],
                            w1[:, (v * DT + d) * H:(v * DT + d + 1) * H])
                for h in range(HT):
                    nc.sync.dma_start(
                        w2sb[:, h, :, :], w2[:, h * 2 * D:(h + 1) * 2 * D])

            def emit_x_load():
                for v in range(2):
                    for d in range(DT):
                        nc.sync.dma_start(
                            xtsb[:, v, d, :],
                            xt[:, (v * DT + d) * C:(v * DT + d + 1) * C])

            def emit_s1():
                # psum = 2*SX*SW1 * (x @ W1) via xH@wH + xL@wH + xH@wL
                TERMS = ((0, 0), (0, 1), (1, 0))   # (w version, x version)
                for h in range(HT):
                    ab = "AB"[h % 2]
                    ps = [psum.tile([128, CTS[c]], f32, name=f"p{ab}{c}",
                                    tag=f"p{ab}{c}", bufs=1) for c in range(NCT)]
                    hc = h * 128
                    seq = [(dp, ti) for ti in range(3) for dp in range(DT // 2)]
                    for si, (dp, ti) in enumerate(seq):
                        dsl = slice(2 * dp, 2 * dp + 2)
                        wv, xv = TERMS[ti]
                        for c in range(NCT):
                            cs = slice(COFF[c], COFF[c] + CTS[c])
                            nc.tensor.matmul(
                                ps[c][:], w1sb[:, wv, dsl, hc:hc + 128],
                                xtsb[:, xv, dsl, cs],
                                start=(si == 0),
                                stop=(si == len(seq) - 1),
                                perf_mode=DR)
                    for c in range(NCT):
                        cs = slice(COFF[c], COFF[c] + CTS[c])
                        nc.scalar.activation(ht[:, h, 0, cs], ps[c][:], relu,
                                             bias=b1sb[:, 2 * h:2 * h + 1],
                                             scale=sA)
                        nc.scalar.activation(ht[:, h, 1, cs], ps[c][:], relu,
                                             bias=b1sb[:, 2 * h + 1:2 * h + 2],
                                             scale=sB)

            def emit_s2():
                # psum = 2*SH*SW2 * (h @ W2) via hA@w2a + hB@w2b per slab
                for d in range(DT):
                    ab = "AB"[d % 2]
                    ps = [psum.tile([128, CTS[c]], f32, name=f"p{ab}{c}",
                                    tag=f"p{ab}{c}", bufs=1) for c in range(NCT)]
                    dc = d * 128
                    for hs in range(HT):
                        for c in range(NCT):
                            cs = slice(COFF[c], COFF[c] + CTS[c])
                            nc.tensor.matmul(
                                ps[c][:], w2sb[:, hs, 0:2, dc:dc + 128],
                                ht[:, hs, 0:2, cs],
                                start=(hs == 0), stop=(hs == HT - 1),
                                perf_mode=DR)
                    for c in range(NCT):
                        yo = persist.tile([128, CTS[c]], f32, name="yo",
                                          tag=f"yo{c % 2}")
                        nc.vector.tensor_copy(yo[:], ps[c][:])
                        nc.scalar.dma_start(
                            yt[dc:dc + 128, COFF[c]:COFF[c] + CTS[c]], yo[:])

            if repeats == 1:
                emit_weight_loads()
                emit_x_load()
                emit_s1()
                emit_s2()
            else:
                emit_weight_loads()
                nc.gpsimd.memset(ht[:], 0.0)
                with tc.For_i(0, repeats, 1,
                              hint_engines=(mybir.EngineType.PE,)):
                    emit_x_load()
                    emit_s2()
                    emit_s1()

    nc.compile()
    return nc


def _get_nc():
    if "nc" not in _CACHE:
        _CACHE["nc"] = _build_bass()
    return _CACHE["nc"]


def _f8(a):
    return np.clip(np.asarray(a, np.float32), -240, 240).astype(FP8)


def _route(x, gate_W, gate_b):
    """float64 gating: (idxs [N,2], gates [N,2]) matching softmax-top2."""
    logits = x.astype(np.float64) @ gate_W.astype(np.float64) + gate_b.astype(np.float64)
    part = np.argpartition(-logits, TOP_K - 1, axis=1)[:, :TOP_K]
    part_vals = np.take_along_axis(logits, part, axis=1)
    order = np.lexsort((part, -part_vals), axis=1)
    idxs = np.take_along_axis(part, order, axis=1)
    m = logits.max(axis=1, keepdims=True)
    ex = np.exp(logits - m)
    probs = ex / ex.sum(axis=1, keepdims=True)
    gates = np.take_along_axis(probs, idxs, axis=1)
    return idxs, gates


def _make_in_maps(x, W1, b1, W2, idxs, gates):
    """Per-core device input dicts + (device rows, overflow rows) per expert."""
    x64 = x.astype(np.float64)
    in_maps, rows_per_e, over_per_e = [], [], []
    for e in range(E):
        rows = np.where((idxs[:, 0] == e) | (idxs[:, 1] == e))[0]
        if len(rows) > C:
            # capacity overflow: keep the C highest-gate tokens on device,
            # compute the rest exactly on host in the combine step.
            g = np.where(idxs[rows, 0] == e, gates[rows, 0], gates[rows, 1])
            sel = np.argsort(-g, kind="stable")
            keep = np.sort(rows[sel[:C]])
            over = np.sort(rows[sel[C:]])
        else:
            keep, over = rows, rows[:0]
        rows_per_e.append(keep)
        over_per_e.append(over)

        xe = np.zeros((C, D))
        xe[: len(keep)] = x64[keep]
        xs = xe.T * SX                                   # [D, C]
        xH = _f8(xs)
        xL = _f8(xs - xH.astype(np.float64))
        xpair = np.stack([xH.reshape(DT, 128, C), xL.reshape(DT, 128, C)],
                         axis=0).transpose(2, 0, 1, 3)   # [128, 2, DT, C]

        ws = W1[e].astype(np.float64) * (2 * SW1)        # [D, H]
        wH = _f8(ws)
        wL = _f8(ws - wH.astype(np.float64))
        w1pair = np.stack([wH.reshape(DT, 128, H), wL.reshape(DT, 128, H)],
                          axis=0).transpose(2, 0, 1, 3)  # [128, 2, DT, H]

        ws2 = W2[e].astype(np.float64) * SW2             # [H, D]
        w2a = _f8(ws2)
        w2b = _f8((2 * ws2 - w2a.astype(np.float64)) / DSHIFT)
        w2pair = np.stack([w2a, w2b], axis=1).reshape(HT, 128, 2, D) \
                   .transpose(1, 0, 2, 3)                # [128, HT, 2, D]

        b1p = np.stack([b1[e] * SH, b1[e] * SH * DSHIFT], axis=1)   # [H, 2]
        b1r = b1p.reshape(HT, 128, 2).transpose(1, 0, 2).reshape(128, 2 * HT)

        in_maps.append({
            "xt": np.ascontiguousarray(xpair.reshape(128, 2 * DT * C)),
            "w1": np.ascontiguousarray(w1pair.reshape(128, 2 * DT * H)),
            "w2": np.ascontiguousarray(w2pair.reshape(128, HT * 2 * D)),
            "b1t": np.ascontiguousarray(b1r.astype(np.float32)),
        })
    return in_maps, rows_per_e, over_per_e


def _get_runner():
    """Compiled SPMD executor (shard_map over 8 cores), cached across calls."""
    if "runner" in _CACHE:
        return _CACHE["runner"]
    import jax
    from jax.sharding import Mesh, PartitionSpec
    from jax.experimental.shard_map import shard_map
    from concourse import mybir
    from concourse.bass2jax import (
        _bass_exec_p, install_neuronx_cc_hook, partition_id_tensor,
    )

    nc = _get_nc()
    install_neuronx_cc_hook()
    partition_name = nc.partition_id_tensor.name if nc.partition_id_tensor else None

    in_names, out_names, out_avals, zero_outs = [], [], [], []
    for alloc in nc.m.functions[0].allocations:
        if not isinstance(alloc, mybir.MemoryLocationSet):
            continue
        name = alloc.memorylocations[0].name
        if alloc.kind == "ExternalInput":
            if name != partition_name:
                in_names.append(name)
        elif alloc.kind == "ExternalOutput":
            out_names.append(name)
            shape, dtype = tuple(alloc.tensor_shape), mybir.dt.np(alloc.dtype)
            out_avals.append(jax.core.ShapedArray(shape, dtype))
            zero_outs.append(np.zeros(shape, dtype))
    n_params = len(in_names)
    all_names = list(in_names) + out_names
    if partition_name is not None:
        all_names.append(partition_name)

    def _body(*args):
        operands = list(args)
        if partition_name is not None:
            operands.append(partition_id_tensor())
        outs = _bass_exec_p.bind(
            *operands, out_avals=tuple(out_avals), in_names=tuple(all_names),
            out_names=tuple(out_names), lowering_input_output_aliases=(),
            sim_require_finite=True, sim_require_nnan=True, nc=nc)
        return tuple(outs)

    devices = jax.devices()[:E]
    mesh = Mesh(np.asarray(devices), ("core",))
    spec = PartitionSpec("core")
    fn = jax.jit(shard_map(
        _body, mesh=mesh,
        in_specs=(spec,) * (n_params + len(out_names)),
        out_specs=(spec,) * len(out_names), check_rep=False))

    def run(in_maps):
        concat = [np.concatenate([np.asarray(m[n]) for m in in_maps], axis=0)
                  for n in in_names]
        concat += [np.concatenate([z] * E, axis=0) for z in zero_outs]
        outs = fn(*concat)
        return [
            {name: np.asarray(outs[i]).reshape(E, *out_avals[i].shape)[c]
             for i, name in enumerate(out_names)}
            for c in range(E)
        ]

    _CACHE["runner"] = run
    return run


def kernel(x, gate_W, gate_b, W1, b1, W2, b2):
    x = np.asarray(x, dtype=np.float32)
    gate_W = np.asarray(gate_W, dtype=np.float32)
    gate_b = np.asarray(gate_b, dtype=np.float32)
    W1 = np.asarray(W1, dtype=np.float32)
    b1 = np.asarray(b1, dtype=np.float32)
    W2 = np.asarray(W2, dtype=np.float32)
    b2 = np.asarray(b2, dtype=np.float32)

    idxs, gates = _route(x, gate_W, gate_b)
    in_maps, rows_per_e, over_per_e = _make_in_maps(x, W1, b1, W2, idxs, gates)

    results = _get_runner()(in_maps)

    out = np.zeros((N, D), dtype=np.float64)
    yscale = 1.0 / (2 * SH * SW2)
    x64 = x.astype(np.float64)
    for e in range(E):
        rows = rows_per_e[e]
        y = results[e]["yt"].T[: len(rows)].astype(np.float64) * yscale \
            + b2[e].astype(np.float64)
        g = np.where(idxs[rows, 0] == e, gates[rows, 0], gates[rows, 1])
        out[rows] += g[:, None] * y
        over = over_per_e[e]
        if len(over):
            # exact host path for over-capacity tokens
            hover = np.maximum(
                x64[over] @ W1[e].astype(np.float64) + b1[e].astype(np.float64), 0.0)
            yover = hover @ W2[e].astype(np.float64) + b2[e].astype(np.float64)
            gover = np.where(idxs[over, 0] == e, gates[over, 0], gates[over, 1])
            out[over] += gover[:, None] * yover
    return out.astype(np.float32)


# revision 8
# speedup vs baseline: 1.2954x; 1.2954x over previous
"""MoE layer (top-2 routing, E=8 experts) on 8 Trainium2 NeuronCores.

Strategy (expert parallelism):
  - Host: gate (float64) -> top-2 routing; per-expert token gather.
  - Device core e: the expert-e MLP over a capacity of C=1024 tokens
    (the mean load), computed with fp8 DoubleRow matmuls (2 contraction
    rows per cycle, ~4x the bf16 PE rate measured on hw) plus error
    compensation to stay inside the 2e-2 gate:
      * Stage 1 (x @ W1): fully hi/lo-compensated fp8. Host splits both
        operands X = xH + xL, W = wH + wL into fp8 pairs; the device
        accumulates xH@wH + xL@wH + xH@wL in PSUM (3 DoubleRow matmuls
        per k-slab pair), giving ~bf16-level precision.
      * Stage 2 (h @ W2): dither-pair fp8. The PSUM h is evicted twice
        (ScalarE relu+scale) as hA = f8(SH*h) and hB = f8(SH*h*DSHIFT)
        on phase-shifted fp8 grids; the host prepares W2 pairs (w2a,
        w2b) where w2b greedily cancels w2a's rounding error. Each
        slab is then ONE DoubleRow matmul hA@w2a + hB@w2b whose noise
        is ~half of naive fp8 per source.
    Measured end-to-end rel err ~1.6e-2 (gate: 2e-2); naive fp8 would
    be 4.3e-2 and pure bf16 2.9e-3 but ~1.4x slower.
  - Tokens over capacity (expert load > 1024, ~1-3% of token-expert
    pairs) are computed exactly on host in float64 during the combine
    (standard capacity-factor expert parallelism, with exact fallback
    instead of token dropping).
  - Chunks are 2x512 columns: 512 is the ISA max matmul output width,
    and per-instruction overhead (~140ns: stationary-load + issue)
    dominates narrower chunks, so minimizing instruction count wins.
  - Timing loop (repeats>1): tc.For_i puts an all-engine barrier at the
    back-edge which would expose the x load; the body is ROTATED
    [load x(k); stage2(k-1); stage1(k)] so stage-2 (whose inputs are
    already SBUF-resident) restarts the PE immediately while x streams.
"""
import sys

sys.path.insert(0, "/opt/trn_rl_repo")

import numpy as np
import ml_dtypes

N, D, H, E, TOP_K = 4096, 1024, 2048, 8, 2
C = 1024            # per-expert device capacity (= mean load)
CTS = (512, 512)    # psum chunk widths (512 = ISA max matmul out)
COFF = (0, 512)
NCT = len(CTS)
DT = D // 128       # 8
HT = H // 128       # 16

FP8 = ml_dtypes.float8_e4m3
SX, SW1, SH, SW2 = 32.0, 1024.0, 32.0, 1024.0   # fp8 pre-scales
DSHIFT = 1.0 + 2.0 ** -4                        # stage-2 dither grid shift

_CACHE = {}
MODE = "C"     # "scheme" (s1 hi/lo comp + s2 dither fp8) | "C" (s1 dither fp8 + s2 bf16)
S2CTS = (512, 512)   # stage-2 psum chunking for MODE C
PSG = 2        # psum group rotation depth
S1_MODE = "scheme"   # "scheme" (hi/lo comp) | "naive" (version-0 only; timing tests)
S2_MODE = "scheme"   # "scheme" (dither pair) | "naive"
SPLIT = 0      # >0: close/reopen psum groups every SPLIT matmuls (hw no-op)


def _build_bass(repeats=1):
    if MODE == "C":
        return _build_bass_c(repeats)
    return _build_bass_scheme(repeats)


def _build_bass_c(repeats=1):
    import concourse.tile as tile
    from concourse import bacc, mybir
    import ml_dtypes as _md

    f32 = mybir.dt.float32
    f8 = mybir.dt.float8e4
    bf16 = mybir.dt.bfloat16
    DR = mybir.MatmulPerfMode.DoubleRow

    nc = bacc.Bacc("TRN2", target_bir_lowering=False, debug=False, num_devices=E)

    # xt/w1: version-inner fp8 dither pairs [d][v][.]; w2/ht: bf16
    xt = nc.dram_tensor("xt", [128, DT * 2 * C], f8, kind="ExternalInput").ap()
    w1 = nc.dram_tensor("w1", [128, DT * 2 * H], f8, kind="ExternalInput").ap()
    w2 = nc.dram_tensor("w2", [128, HT * D], bf16, kind="ExternalInput").ap()
    b1t = nc.dram_tensor("b1t", [128, HT], f32, kind="ExternalInput").ap()
    yt = nc.dram_tensor("yt", [D, C], f32, kind="ExternalOutput").ap()

    relu = mybir.ActivationFunctionType.Relu
    s_act = 1.0 / (2 * SX * SW1)
    CT2 = S2CTS
    CO2 = tuple(int(v) for v in np.cumsum((0,) + S2CTS)[:-1])

    with tile.TileContext(nc) as tc:
        with (
            tc.tile_pool(name="persist", bufs=1) as persist,
            tc.tile_pool(name="psum", bufs=2, space="PSUM") as psum,
        ):
            b1sb = persist.tile([128, HT], f32, name="b1sb", tag="b1")
            w1sb = persist.tile([128, DT, 2, H], f8, name="w1sb", tag="w1")
            xtsb = persist.tile([128, DT, 2, C], f8, name="xtsb", tag="xt")
            w2sb = persist.tile([128, HT, D], bf16, name="w2sb", tag="w2")
            ht = persist.tile([128, HT, C], bf16, name="ht", tag="ht")

            def emit_weight_loads():
                nc.sync.dma_start(b1sb[:], b1t[:])
                for d in range(DT):
                    nc.sync.dma_start(w1sb[:, d, :, :],
                                      w1[:, d * 2 * H:(d + 1) * 2 * H])
                for h in range(HT):
                    nc.sync.dma_start(w2sb[:, h, :], w2[:, h * D:(h + 1) * D])

            def emit_x_load():
                for d in range(DT):
                    nc.sync.dma_start(xtsb[:, d, :, :],
                                      xt[:, d * 2 * C:(d + 1) * 2 * C])

            def emit_s1():
                # psum = 2*SX*SW1 * x@W1 via dither pair xa@wa + xb@wb per slab
                for h in range(HT):
                    ab = "ABC"[h % PSG]
                    ps = [psum.tile([128, CTS[c]], f32, name=f"p{ab}{c}",
                                    tag=f"p{ab}{c}", bufs=1) for c in range(NCT)]
                    hc = h * 128
                    for d in range(DT):
                        for c in range(NCT):
                            cs = slice(COFF[c], COFF[c] + CTS[c])
                            nc.tensor.matmul(
                                ps[c][:], w1sb[:, d, 0:2, hc:hc + 128],
                                xtsb[:, d, 0:2, cs],
                                start=(d == 0), stop=(d == DT - 1),
                                perf_mode=DR)
                    for c in range(NCT):
                        cs = slice(COFF[c], COFF[c] + CTS[c])
                        nc.scalar.activation(ht[:, h, cs], ps[c][:], relu,
                                             bias=b1sb[:, h:h + 1], scale=s_act)

            def emit_s2():
                # y = h @ W2 in bf16
                for d in range(DT):
                    ab = "ABC"[d % PSG]
                    ps = [psum.tile([128, CT2[c]], f32, name=f"p{ab}{c}",
                                    tag=f"p{ab}{c}", bufs=1)
                          for c in range(len(CT2))]
                    dc = d * 128
                    for hs in range(HT):
                        for c in range(len(CT2)):
                            cs = slice(CO2[c], CO2[c] + CT2[c])
                            nc.tensor.matmul(
                                ps[c][:], w2sb[:, hs, dc:dc + 128],
                                ht[:, hs, cs],
                                start=(hs == 0), stop=(hs == HT - 1))
                    for c in range(len(CT2)):
                        yo = persist.tile([128, CT2[c]], f32, name="yo",
                                          tag=f"yo{c % 2}")
                        nc.vector.tensor_copy(yo[:], ps[c][:])
                        nc.scalar.dma_start(
                            yt[dc:dc + 128, CO2[c]:CO2[c] + CT2[c]], yo[:])

            if repeats == 1:
                emit_weight_loads()
                emit_x_load()
                emit_s1()
                emit_s2()
            else:
                emit_weight_loads()
                nc.gpsimd.memset(ht[:], 0.0)
                with tc.For_i(0, repeats, 1,
                              hint_engines=(mybir.EngineType.PE,)):
                    emit_x_load()
                    emit_s2()
                    emit_s1()

    nc.compile()
    return nc


def _build_bass_scheme(repeats=1):
    import concourse.tile as tile
    from concourse import bacc, mybir

    f32 = mybir.dt.float32
    f8 = mybir.dt.float8e4
    DR = mybir.MatmulPerfMode.DoubleRow

    nc = bacc.Bacc("TRN2", target_bir_lowering=False, debug=False, num_devices=E)

    # Host-prearranged SBUF layouts, partition dim first:
    #   xt: [128, 2(ver) * DT * C]   ver 0 = xH, 1 = xL  (fp8, scaled SX)
    #   w1: [128, 2(ver) * DT * H]   ver 0 = wH, 1 = wL  (fp8, scaled 2*SW1)
    #   w2: [128, HT * 2(ver) * D]   ver 0 = w2a, 1 = w2b (fp8, scaled SW2)
    #   b1t:[128, 2 * HT]            col 2h+v = bias for version v of h-tile
    xt = nc.dram_tensor("xt", [128, 2 * DT * C], f8, kind="ExternalInput").ap()
    w1 = nc.dram_tensor("w1", [128, 2 * DT * H], f8, kind="ExternalInput").ap()
    w2 = nc.dram_tensor("w2", [128, HT * 2 * D], f8, kind="ExternalInput").ap()
    b1t = nc.dram_tensor("b1t", [128, 2 * HT], f32, kind="ExternalInput").ap()
    yt = nc.dram_tensor("yt", [D, C], f32, kind="ExternalOutput").ap()

    relu = mybir.ActivationFunctionType.Relu
    sA = SH / (2 * SX * SW1)      # psum -> SH*h eviction scale
    sB = sA * DSHIFT

    with tile.TileContext(nc) as tc:
        with (
            tc.tile_pool(name="persist", bufs=1) as persist,
            tc.tile_pool(name="psum", bufs=2, space="PSUM") as psum,
        ):
            b1sb = persist.tile([128, 2 * HT], f32, name="b1sb", tag="b1")
            w1sb = persist.tile([128, 2, DT, H], f8, name="w1sb", tag="w1")
            xtsb = persist.tile([128, 2, DT, C], f8, name="xtsb", tag="xt")
            w2sb = persist.tile([128, HT, 2, D], f8, name="w2sb", tag="w2")
            ht = persist.tile([128, HT, 2, C], f8, name="ht", tag="ht")

            def emit_weight_loads():
                # Static weights: loaded once, SBUF-resident across calls.
                nc.sync.dma_start(b1sb[:], b1t[:])
                for v in range(2):
                    for d in range(DT):
                        nc.sync.dma_start(
                            w1sb[:, v, d, :],
                            w1[:, (v * DT + d) * H:(v * DT + d + 1) * H])
                for h in range(HT):
                    nc.sync.dma_start(
                        w2sb[:, h, :, :], w2[:, h * 2 * D:(h + 1) * 2 * D])

            def emit_x_load():
                for v in range(2):
                    for d in range(DT):
                        nc.sync.dma_start(
                            xtsb[:, v, d, :],
                            xt[:, (v * DT + d) * C:(v * DT + d + 1) * C])

            def emit_s1():
                # psum = 2*SX*SW1 * (x @ W1) via xH@wH + xL@wH + xH@wL
                if S1_MODE == "scheme":
                    TERMS = ((0, 0), (0, 1), (1, 0))   # (w version, x version)
                else:
                    TERMS = ((0, 0),)
                for h in range(HT):
                    ab = "ABC"[h % PSG]
                    ps = [psum.tile([128, CTS[c]], f32, name=f"p{ab}{c}",
                                    tag=f"p{ab}{c}", bufs=1) for c in range(NCT)]
                    hc = h * 128
                    seq = [(dp, ti) for ti in range(len(TERMS))
                           for dp in range(DT // 2)]
                    nseq = len(seq)
                    for si, (dp, ti) in enumerate(seq):
                        dsl = slice(2 * dp, 2 * dp + 2)
                        wv, xv = TERMS[ti]
                        st = (si == 0)
                        sp = (si == nseq - 1)
                        if SPLIT:
                            sp = sp or (si % SPLIT == SPLIT - 1)
                        for c in range(NCT):
                            cs = slice(COFF[c], COFF[c] + CTS[c])
                            nc.tensor.matmul(
                                ps[c][:], w1sb[:, wv, dsl, hc:hc + 128],
                                xtsb[:, xv, dsl, cs],
                                start=st, stop=sp,
                                skip_group_check=bool(SPLIT),
                                perf_mode=DR)
                    for c in range(NCT):
                        cs = slice(COFF[c], COFF[c] + CTS[c])
                        nc.scalar.activation(ht[:, h, 0, cs], ps[c][:], relu,
                                             bias=b1sb[:, 2 * h:2 * h + 1],
                                             scale=sA)
                        if S2_MODE == "scheme":
                            nc.scalar.activation(ht[:, h, 1, cs], ps[c][:], relu,
                                                 bias=b1sb[:, 2 * h + 1:2 * h + 2],
                                                 scale=sB)

            def emit_s2():
                # psum = 2*SH*SW2 * (h @ W2) via hA@w2a + hB@w2b per slab
                for d in range(DT):
                    ab = "ABC"[d % PSG]
                    ps = [psum.tile([128, CTS[c]], f32, name=f"p{ab}{c}",
                                    tag=f"p{ab}{c}", bufs=1) for c in range(NCT)]
                    dc = d * 128
                    if S2_MODE == "scheme":
                        for hs in range(HT):
                            sp = (hs == HT - 1)
                            if SPLIT:
                                sp = sp or (hs % SPLIT == SPLIT - 1)
                            for c in range(NCT):
                                cs = slice(COFF[c], COFF[c] + CTS[c])
                                nc.tensor.matmul(
                                    ps[c][:], w2sb[:, hs, 0:2, dc:dc + 128],
                                    ht[:, hs, 0:2, cs],
                                    start=(hs == 0), stop=sp,
                                    skip_group_check=bool(SPLIT),
                                    perf_mode=DR)
                    else:
                        for hp in range(HT // 2):
                            for c in range(NCT):
                                cs = slice(COFF[c], COFF[c] + CTS[c])
                                nc.tensor.matmul(
                                    ps[c][:],
                                    w2sb[:, 2 * hp:2 * hp + 2, 0, dc:dc + 128],
                                    ht[:, 2 * hp:2 * hp + 2, 0, cs],
                                    start=(hp == 0), stop=(hp == HT // 2 - 1),
                                    perf_mode=DR)
                    for c in range(NCT):
                        yo = persist.tile([128, CTS[c]], f32, name="yo",
                                          tag=f"yo{c % 2}")
                        nc.vector.tensor_copy(yo[:], ps[c][:])
                        nc.scalar.dma_start(
                            yt[dc:dc + 128, COFF[c]:COFF[c] + CTS[c]], yo[:])

            if repeats == 1:
                emit_weight_loads()
                emit_x_load()
                emit_s1()
                emit_s2()
            else:
                emit_weight_loads()
                nc.gpsimd.memset(ht[:], 0.0)
                with tc.For_i(0, repeats, 1,
                              hint_engines=(mybir.EngineType.PE,)):
                    emit_x_load()
                    emit_s2()
                    emit_s1()

    nc.compile()
    return nc


def _get_nc():
    if "nc" not in _CACHE:
        _CACHE["nc"] = _build_bass()
    return _CACHE["nc"]


def _f8(a):
    return np.clip(np.asarray(a, np.float32), -240, 240).astype(FP8)


def _route(x, gate_W, gate_b):
    """float64 gating: (idxs [N,2], gates [N,2]) matching softmax-top2."""
    logits = x.astype(np.float64) @ gate_W.astype(np.float64) + gate_b.astype(np.float64)
    part = np.argpartition(-logits, TOP_K - 1, axis=1)[:, :TOP_K]
    part_vals = np.take_along_axis(logits, part, axis=1)
    order = np.lexsort((part, -part_vals), axis=1)
    idxs = np.take_along_axis(part, order, axis=1)
    m = logits.max(axis=1, keepdims=True)
    ex = np.exp(logits - m)
    probs = ex / ex.sum(axis=1, keepdims=True)
    gates = np.take_along_axis(probs, idxs, axis=1)
    return idxs, gates


def _make_in_maps(x, W1, b1, W2, idxs, gates):
    if MODE == "C":
        return _make_in_maps_c(x, W1, b1, W2, idxs, gates)
    return _make_in_maps_scheme(x, W1, b1, W2, idxs, gates)


def _make_in_maps_c(x, W1, b1, W2, idxs, gates):
    BF16 = ml_dtypes.bfloat16
    x64 = x.astype(np.float64)
    in_maps, rows_per_e, over_per_e = [], [], []
    for e in range(E):
        rows = np.where((idxs[:, 0] == e) | (idxs[:, 1] == e))[0]
        if len(rows) > C:
            g = np.where(idxs[rows, 0] == e, gates[rows, 0], gates[rows, 1])
            sel = np.argsort(-g, kind="stable")
            keep = np.sort(rows[sel[:C]])
            over = np.sort(rows[sel[C:]])
        else:
            keep, over = rows, rows[:0]
        rows_per_e.append(keep)
        over_per_e.append(over)

        xe = np.zeros((C, D))
        xe[: len(keep)] = x64[keep]
        xs = xe.T * SX
        xa = _f8(xs)
        xb = _f8(2 * xs - xa.astype(np.float64))
        xpair = np.stack([xa.reshape(DT, 128, C), xb.reshape(DT, 128, C)],
                         axis=2).transpose(1, 0, 2, 3)   # [128, DT, 2, C]

        ws = W1[e].astype(np.float64) * SW1
        wa = _f8(ws)
        wb = _f8(2 * ws - wa.astype(np.float64))
        w1pair = np.stack([wa.reshape(DT, 128, H), wb.reshape(DT, 128, H)],
                          axis=2).transpose(1, 0, 2, 3)  # [128, DT, 2, H]

        w2r = W2[e].reshape(HT, 128, D).transpose(1, 0, 2).reshape(128, HT * D)
        b1r = b1[e].reshape(HT, 128).T

        in_maps.append({
            "xt": np.ascontiguousarray(xpair.reshape(128, DT * 2 * C)),
            "w1": np.ascontiguousarray(w1pair.reshape(128, DT * 2 * H)),
            "w2": np.ascontiguousarray(w2r.astype(BF16)),
            "b1t": np.ascontiguousarray(b1r.astype(np.float32)),
        })
    return in_maps, rows_per_e, over_per_e


def _make_in_maps_scheme(x, W1, b1, W2, idxs, gates):
    """Per-core device input dicts + (device rows, overflow rows) per expert."""
    x64 = x.astype(np.float64)
    in_maps, rows_per_e, over_per_e = [], [], []
    for e in range(E):
        rows = np.where((idxs[:, 0] == e) | (idxs[:, 1] == e))[0]
        if len(rows) > C:
            # capacity overflow: keep the C highest-gate tokens on device,
            # compute the rest exactly on host in the combine step.
            g = np.where(idxs[rows, 0] == e, gates[rows, 0], gates[rows, 1])
            sel = np.argsort(-g, kind="stable")
            keep = np.sort(rows[sel[:C]])
            over = np.sort(rows[sel[C:]])
        else:
            keep, over = rows, rows[:0]
        rows_per_e.append(keep)
        over_per_e.append(over)

        xe = np.zeros((C, D))
        xe[: len(keep)] = x64[keep]
        xs = xe.T * SX                                   # [D, C]
        xH = _f8(xs)
        xL = _f8(xs - xH.astype(np.float64))
        xpair = np.stack([xH.reshape(DT, 128, C), xL.reshape(DT, 128, C)],
                         axis=0).transpose(2, 0, 1, 3)   # [128, 2, DT, C]

        ws = W1[e].astype(np.float64) * (2 * SW1)        # [D, H]
        wH = _f8(ws)
        wL = _f8(ws - wH.astype(np.float64))
        w1pair = np.stack([wH.reshape(DT, 128, H), wL.reshape(DT, 128, H)],
                          axis=0).transpose(2, 0, 1, 3)  # [128, 2, DT, H]

        ws2 = W2[e].astype(np.float64) * SW2             # [H, D]
        w2a = _f8(ws2)
        w2b = _f8((2 * ws2 - w2a.astype(np.float64)) / DSHIFT)
        w2pair = np.stack([w2a, w2b], axis=1).reshape(HT, 128, 2, D) \
                   .transpose(1, 0, 2, 3)                # [128, HT, 2, D]

        b1p = np.stack([b1[e] * SH, b1[e] * SH * DSHIFT], axis=1)   # [H, 2]
        b1r = b1p.reshape(HT, 128, 2).transpose(1, 0, 2).reshape(128, 2 * HT)

        in_maps.append({
            "xt": np.ascontiguousarray(xpair.reshape(128, 2 * DT * C)),
            "w1": np.ascontiguousarray(w1pair.reshape(128, 2 * DT * H)),
            "w2": np.ascontiguousarray(w2pair.reshape(128, HT * 2 * D)),
            "b1t": np.ascontiguousarray(b1r.astype(np.float32)),
        })
    return in_maps, rows_per_e, over_per_e


def _get_runner():
    """Compiled SPMD executor (shard_map over 8 cores), cached across calls."""
    if "runner" in _CACHE:
        return _CACHE["runner"]
    import jax
    from jax.sharding import Mesh, PartitionSpec
    from jax.experimental.shard_map import shard_map
    from concourse import mybir
    from concourse.bass2jax import (
        _bass_exec_p, install_neuronx_cc_hook, partition_id_tensor,
    )

    nc = _get_nc()
    install_neuronx_cc_hook()
    partition_name = nc.partition_id_tensor.name if nc.partition_id_tensor else None

    in_names, out_names, out_avals, zero_outs = [], [], [], []
    for alloc in nc.m.functions[0].allocations:
        if not isinstance(alloc, mybir.MemoryLocationSet):
            continue
        name = alloc.memorylocations[0].name
        if alloc.kind == "ExternalInput":
            if name != partition_name:
                in_names.append(name)
        elif alloc.kind == "ExternalOutput":
            out_names.append(name)
            shape, dtype = tuple(alloc.tensor_shape), mybir.dt.np(alloc.dtype)
            out_avals.append(jax.core.ShapedArray(shape, dtype))
            zero_outs.append(np.zeros(shape, dtype))
    n_params = len(in_names)
    all_names = list(in_names) + out_names
    if partition_name is not None:
        all_names.append(partition_name)

    def _body(*args):
        operands = list(args)
        if partition_name is not None:
            operands.append(partition_id_tensor())
        outs = _bass_exec_p.bind(
            *operands, out_avals=tuple(out_avals), in_names=tuple(all_names),
            out_names=tuple(out_names), lowering_input_output_aliases=(),
            sim_require_finite=True, sim_require_nnan=True, nc=nc)
        return tuple(outs)

    devices = jax.devices()[:E]
    mesh = Mesh(np.asarray(devices), ("core",))
    spec = PartitionSpec("core")
    fn = jax.jit(shard_map(
        _body, mesh=mesh,
        in_specs=(spec,) * (n_params + len(out_names)),
        out_specs=(spec,) * len(out_names), check_rep=False))

    def run(in_maps):
        concat = [np.concatenate([np.asarray(m[n]) for m in in_maps], axis=0)
                  for n in in_names]
        concat += [np.concatenate([z] * E, axis=0) for z in zero_outs]
        outs = fn(*concat)
        return [
            {name: np.asarray(outs[i]).reshape(E, *out_avals[i].shape)[c]
             for i, name in enumerate(out_names)}
            for c in range(E)
        ]

    _CACHE["runner"] = run
    return run


def kernel(x, gate_W, gate_b, W1, b1, W2, b2):
    x = np.asarray(x, dtype=np.float32)
    gate_W = np.asarray(gate_W, dtype=np.float32)
    gate_b = np.asarray(gate_b, dtype=np.float32)
    W1 = np.asarray(W1, dtype=np.float32)
    b1 = np.asarray(b1, dtype=np.float32)
    W2 = np.asarray(W2, dtype=np.float32)
    b2 = np.asarray(b2, dtype=np.float32)

    idxs, gates = _route(x, gate_W, gate_b)
    in_maps, rows_per_e, over_per_e = _make_in_maps(x, W1, b1, W2, idxs, gates)

    results = _get_runner()(in_maps)

    out = np.zeros((N, D), dtype=np.float64)
    yscale = 1.0 if MODE == "C" else 1.0 / (2 * SH * SW2)
    x64 = x.astype(np.float64)
    for e in range(E):
        rows = rows_per_e[e]
        y = results[e]["yt"].T[: len(rows)].astype(np.float64) * yscale \
            + b2[e].astype(np.float64)
        g = np.where(idxs[rows, 0] == e, gates[rows, 0], gates[rows, 1])
        out[rows] += g[:, None] * y
        over = over_per_e[e]
        if len(over):
            # exact host path for over-capacity tokens
            hover = np.maximum(
                x64[over] @ W1[e].astype(np.float64) + b1[e].astype(np.float64), 0.0)
            yover = hover @ W2[e].astype(np.float64) + b2[e].astype(np.float64)
            gover = np.where(idxs[over, 0] == e, gates[over, 0], gates[over, 1])
            out[over] += gover[:, None] * yover
    return out.astype(np.float32)


# revision 9
# speedup vs baseline: 1.4572x; 1.1249x over previous
"""MoE layer (top-2 routing, E=8 experts) on 8 Trainium2 NeuronCores.

Strategy (expert parallelism, MODE="C" default):
  - Host: gate (float64) -> top-2 routing; per-expert token gather.
  - Device core e: expert-e MLP over a capacity of C=1024 tokens (the
    mean load; tokens over capacity -- expert load > 1024, ~1-3% of
    token-expert pairs -- are computed exactly on host during the
    combine, i.e. capacity-factor expert parallelism with an exact
    fallback instead of token dropping).
  - Stage 1 (x @ W1): dither-pair fp8 DoubleRow. The host builds two
    jointly-chosen fp8 quantizations of each operand (b = f8(2a - f8(a))
    greedy completion, equivalent to quantizing on the half-step
    midpoint grid) and the device sums xa@wa + xb@wb in one DoubleRow
    matmul per k-slab (slots carry the two versions), halving fp8
    noise per source at 2x the ideal bf16 PE rate.  DoubleRow feeds
    2 contraction rows/cycle (measured ~4x bf16 per instruction), but
    per-instruction overhead (~140-250ns: stationary load + issue +
    accumulation turnaround) makes instruction count the real cost;
    the dither pair needs only 1 instruction per k-slab (vs 2 for
    bf16-equivalent full hi/lo compensation).
  - Stage 2 (h @ W2): plain bf16 (h evicted once as bf16, W2 bf16).
    Stage-2 in fp8 would need noise compensation that costs as many
    instructions as bf16 streams; bf16 is simpler and noise-free.
  - Measured end-to-end rel err 1.68e-2 (gate: 2e-2). All-bf16 gives
    2.9e-3 but is ~1.2-1.4x slower; naive all-fp8 gives 4.3e-2 (fail).
  - Chunks are 2x512 columns: 512 is the ISA max matmul output width
    (s3d3_mm_num_elements), and a 3rd chunk would add +50% matmul
    instructions at fixed per-instruction overhead.  PSUM tags are
    shared between stages (4 tiles x2 rotation = 8 banks exactly).
  - Timing loop (repeats>1): tc.For_i puts an all-engine barrier at the
    back-edge which would expose the x load; the body is ROTATED
    [load x(k); stage2(k-1); stage1(k)] so stage-2 (whose inputs are
    already SBUF-resident) restarts the PE immediately while x streams.

MODE="scheme" keeps the alternative all-fp8 design (stage-1 full hi/lo
compensation + stage-2 dither, rel err 1.63e-2): fewer PE data-cycles
on paper but more instructions (640 vs 512), measured slower.
"""
import sys

sys.path.insert(0, "/opt/trn_rl_repo")

import numpy as np
import ml_dtypes

N, D, H, E, TOP_K = 4096, 1024, 2048, 8, 2
C = 1024            # per-expert device capacity (= mean load)
CTS = (512, 512)    # psum chunk widths (512 = ISA max matmul out)
COFF = (0, 512)
NCT = len(CTS)
DT = D // 128       # 8
HT = H // 128       # 16

FP8 = ml_dtypes.float8_e4m3
SX, SW1, SH, SW2 = 32.0, 1024.0, 32.0, 1024.0   # fp8 pre-scales
DSHIFT = 1.0 + 2.0 ** -4                        # stage-2 dither grid shift

_CACHE = {}
MODE = "C"     # "scheme" (s1 hi/lo comp + s2 dither fp8) | "C" (s1 dither fp8 + s2 bf16)
S2CTS = (512, 512)   # stage-2 psum chunking for MODE C
PSG = 2        # psum group rotation depth
S1_MODE = "scheme"   # "scheme" (hi/lo comp) | "naive" (version-0 only; timing tests)
S2_MODE = "scheme"   # "scheme" (dither pair) | "naive"
SPLIT = 0      # >0: close/reopen psum groups every SPLIT matmuls (hw no-op)


def _build_bass(repeats=1):
    if MODE == "C":
        return _build_bass_c(repeats)
    return _build_bass_scheme(repeats)


def _build_bass_c(repeats=1):
    import concourse.tile as tile
    from concourse import bacc, mybir
    import ml_dtypes as _md

    f32 = mybir.dt.float32
    f8 = mybir.dt.float8e4
    bf16 = mybir.dt.bfloat16
    DR = mybir.MatmulPerfMode.DoubleRow

    nc = bacc.Bacc("TRN2", target_bir_lowering=False, debug=False, num_devices=E)

    # xt/w1: version-inner fp8 dither pairs [d][v][.]; w2/ht: bf16
    xt = nc.dram_tensor("xt", [128, DT * 2 * C], f8, kind="ExternalInput").ap()
    w1 = nc.dram_tensor("w1", [128, DT * 2 * H], f8, kind="ExternalInput").ap()
    w2 = nc.dram_tensor("w2", [128, HT * D], bf16, kind="ExternalInput").ap()
    b1t = nc.dram_tensor("b1t", [128, HT], f32, kind="ExternalInput").ap()
    yt = nc.dram_tensor("yt", [D, C], f32, kind="ExternalOutput").ap()

    relu = mybir.ActivationFunctionType.Relu
    s_act = 1.0 / (2 * SX * SW1)
    CT2 = S2CTS
    CO2 = tuple(int(v) for v in np.cumsum((0,) + S2CTS)[:-1])

    with tile.TileContext(nc) as tc:
        with (
            tc.tile_pool(name="persist", bufs=1) as persist,
            tc.tile_pool(name="psum", bufs=2, space="PSUM") as psum,
        ):
            b1sb = persist.tile([128, HT], f32, name="b1sb", tag="b1")
            w1sb = persist.tile([128, DT, 2, H], f8, name="w1sb", tag="w1")
            xtsb = persist.tile([128, DT, 2, C], f8, name="xtsb", tag="xt")
            w2sb = persist.tile([128, HT, D], bf16, name="w2sb", tag="w2")
            ht = persist.tile([128, HT, C], bf16, name="ht", tag="ht")

            def emit_weight_loads():
                nc.sync.dma_start(b1sb[:], b1t[:])
                for d in range(DT):
                    nc.sync.dma_start(w1sb[:, d, :, :],
                                      w1[:, d * 2 * H:(d + 1) * 2 * H])
                for h in range(HT):
                    nc.sync.dma_start(w2sb[:, h, :], w2[:, h * D:(h + 1) * D])

            def emit_x_load():
                for d in range(DT):
                    nc.sync.dma_start(xtsb[:, d, :, :],
                                      xt[:, d * 2 * C:(d + 1) * 2 * C])

            def emit_s1():
                # psum = 2*SX*SW1 * x@W1 via dither pair xa@wa + xb@wb per slab
                for h in range(HT):
                    ab = "ABC"[h % PSG]
                    ps = [psum.tile([128, CTS[c]], f32, name=f"p{ab}{c}",
                                    tag=f"p{ab}{c}", bufs=1) for c in range(NCT)]
                    hc = h * 128
                    for d in range(DT):
                        for c in range(NCT):
                            cs = slice(COFF[c], COFF[c] + CTS[c])
                            nc.tensor.matmul(
                                ps[c][:], w1sb[:, d, 0:2, hc:hc + 128],
                                xtsb[:, d, 0:2, cs],
                                start=(d == 0), stop=(d == DT - 1),
                                perf_mode=DR)
                    for c in range(NCT):
                        cs = slice(COFF[c], COFF[c] + CTS[c])
                        nc.scalar.activation(ht[:, h, cs], ps[c][:], relu,
                                             bias=b1sb[:, h:h + 1], scale=s_act)

            def emit_s2():
                # y = h @ W2 in bf16
                for d in range(DT):
                    ab = "ABC"[d % PSG]
                    ps = [psum.tile([128, CT2[c]], f32, name=f"p{ab}{c}",
                                    tag=f"p{ab}{c}", bufs=1)
                          for c in range(len(CT2))]
                    dc = d * 128
                    for hs in range(HT):
                        for c in range(len(CT2)):
                            cs = slice(CO2[c], CO2[c] + CT2[c])
                            nc.tensor.matmul(
                                ps[c][:], w2sb[:, hs, dc:dc + 128],
                                ht[:, hs, cs],
                                start=(hs == 0), stop=(hs == HT - 1))
                    for c in range(len(CT2)):
                        yo = persist.tile([128, CT2[c]], f32, name="yo",
                                          tag=f"yo{c % 2}")
                        nc.vector.tensor_copy(yo[:], ps[c][:])
                        nc.scalar.dma_start(
                            yt[dc:dc + 128, CO2[c]:CO2[c] + CT2[c]], yo[:])

            if repeats == 1:
                emit_weight_loads()
                emit_x_load()
                emit_s1()
                emit_s2()
            else:
                emit_weight_loads()
                nc.gpsimd.memset(ht[:], 0.0)
                with tc.For_i(0, repeats, 1,
                              hint_engines=(mybir.EngineType.PE,)):
                    emit_x_load()
                    emit_s2()
                    emit_s1()

    nc.compile()
    return nc


def _build_bass_scheme(repeats=1):
    import concourse.tile as tile
    from concourse import bacc, mybir

    f32 = mybir.dt.float32
    f8 = mybir.dt.float8e4
    DR = mybir.MatmulPerfMode.DoubleRow

    nc = bacc.Bacc("TRN2", target_bir_lowering=False, debug=False, num_devices=E)

    # Host-prearranged SBUF layouts, partition dim first:
    #   xt: [128, 2(ver) * DT * C]   ver 0 = xH, 1 = xL  (fp8, scaled SX)
    #   w1: [128, 2(ver) * DT * H]   ver 0 = wH, 1 = wL  (fp8, scaled 2*SW1)
    #   w2: [128, HT * 2(ver) * D]   ver 0 = w2a, 1 = w2b (fp8, scaled SW2)
    #   b1t:[128, 2 * HT]            col 2h+v = bias for version v of h-tile
    xt = nc.dram_tensor("xt", [128, 2 * DT * C], f8, kind="ExternalInput").ap()
    w1 = nc.dram_tensor("w1", [128, 2 * DT * H], f8, kind="ExternalInput").ap()
    w2 = nc.dram_tensor("w2", [128, HT * 2 * D], f8, kind="ExternalInput").ap()
    b1t = nc.dram_tensor("b1t", [128, 2 * HT], f32, kind="ExternalInput").ap()
    yt = nc.dram_tensor("yt", [D, C], f32, kind="ExternalOutput").ap()

    relu = mybir.ActivationFunctionType.Relu
    sA = SH / (2 * SX * SW1)      # psum -> SH*h eviction scale
    sB = sA * DSHIFT

    with tile.TileContext(nc) as tc:
        with (
            tc.tile_pool(name="persist", bufs=1) as persist,
            tc.tile_pool(name="psum", bufs=2, space="PSUM") as psum,
        ):
            b1sb = persist.tile([128, 2 * HT], f32, name="b1sb", tag="b1")
            w1sb = persist.tile([128, 2, DT, H], f8, name="w1sb", tag="w1")
            xtsb = persist.tile([128, 2, DT, C], f8, name="xtsb", tag="xt")
            w2sb = persist.tile([128, HT, 2, D], f8, name="w2sb", tag="w2")
            ht = persist.tile([128, HT, 2, C], f8, name="ht", tag="ht")

            def emit_weight_loads():
                # Static weights: loaded once, SBUF-resident across calls.
                nc.sync.dma_start(b1sb[:], b1t[:])
                for v in range(2):
                    for d in range(DT):
                        nc.sync.dma_start(
                            w1sb[:, v, d, :],
                            w1[:, (v * DT + d) * H:(v * DT + d + 1) * H])
                for h in range(HT):
                    nc.sync.dma_start(
                        w2sb[:, h, :, :], w2[:, h * 2 * D:(h + 1) * 2 * D])

            def emit_x_load():
                for v in range(2):
                    for d in range(DT):
                        nc.sync.dma_start(
                            xtsb[:, v, d, :],
                            xt[:, (v * DT + d) * C:(v * DT + d + 1) * C])

            def emit_s1():
                # psum = 2*SX*SW1 * (x @ W1) via xH@wH + xL@wH + xH@wL
                if S1_MODE == "scheme":
                    TERMS = ((0, 0), (0, 1), (1, 0))   # (w version, x version)
                else:
                    TERMS = ((0, 0),)
                for h in range(HT):
                    ab = "ABC"[h % PSG]
                    ps = [psum.tile([128, CTS[c]], f32, name=f"p{ab}{c}",
                                    tag=f"p{ab}{c}", bufs=1) for c in range(NCT)]
                    hc = h * 128
                    seq = [(dp, ti) for ti in range(len(TERMS))
                           for dp in range(DT // 2)]
                    nseq = len(seq)
                    for si, (dp, ti) in enumerate(seq):
                        dsl = slice(2 * dp, 2 * dp + 2)
                        wv, xv = TERMS[ti]
                        st = (si == 0)
                        sp = (si == nseq - 1)
                        if SPLIT:
                            sp = sp or (si % SPLIT == SPLIT - 1)
                        for c in range(NCT):
                            cs = slice(COFF[c], COFF[c] + CTS[c])
                            nc.tensor.matmul(
                                ps[c][:], w1sb[:, wv, dsl, hc:hc + 128],
                                xtsb[:, xv, dsl, cs],
                                start=st, stop=sp,
                                skip_group_check=bool(SPLIT),
                                perf_mode=DR)
                    for c in range(NCT):
                        cs = slice(COFF[c], COFF[c] + CTS[c])
                        nc.scalar.activation(ht[:, h, 0, cs], ps[c][:], relu,
                                             bias=b1sb[:, 2 * h:2 * h + 1],
                                             scale=sA)
                        if S2_MODE == "scheme":
                            nc.scalar.activation(ht[:, h, 1, cs], ps[c][:], relu,
                                                 bias=b1sb[:, 2 * h + 1:2 * h + 2],
                                                 scale=sB)

            def emit_s2():
                # psum = 2*SH*SW2 * (h @ W2) via hA@w2a + hB@w2b per slab
                for d in range(DT):
                    ab = "ABC"[d % PSG]
                    ps = [psum.tile([128, CTS[c]], f32, name=f"p{ab}{c}",
                                    tag=f"p{ab}{c}", bufs=1) for c in range(NCT)]
                    dc = d * 128
                    if S2_MODE == "scheme":
                        for hs in range(HT):
                            sp = (hs == HT - 1)
                            if SPLIT:
                                sp = sp or (hs % SPLIT == SPLIT - 1)
                            for c in range(NCT):
                                cs = slice(COFF[c], COFF[c] + CTS[c])
                                nc.tensor.matmul(
                                    ps[c][:], w2sb[:, hs, 0:2, dc:dc + 128],
                                    ht[:, hs, 0:2, cs],
                                    start=(hs == 0), stop=sp,
                                    skip_group_check=bool(SPLIT),
                                    perf_mode=DR)
                    else:
                        for hp in range(HT // 2):
                            for c in range(NCT):
                                cs = slice(COFF[c], COFF[c] + CTS[c])
                                nc.tensor.matmul(
                                    ps[c][:],
                                    w2sb[:, 2 * hp:2 * hp + 2, 0, dc:dc + 128],
                                    ht[:, 2 * hp:2 * hp + 2, 0, cs],
                                    start=(hp == 0), stop=(hp == HT // 2 - 1),
                                    perf_mode=DR)
                    for c in range(NCT):
                        yo = persist.tile([128, CTS[c]], f32, name="yo",
                                          tag=f"yo{c % 2}")
                        nc.vector.tensor_copy(yo[:], ps[c][:])
                        nc.scalar.dma_start(
                            yt[dc:dc + 128, COFF[c]:COFF[c] + CTS[c]], yo[:])

            if repeats == 1:
                emit_weight_loads()
                emit_x_load()
                emit_s1()
                emit_s2()
            else:
                emit_weight_loads()
                nc.gpsimd.memset(ht[:], 0.0)
                with tc.For_i(0, repeats, 1,
                              hint_engines=(mybir.EngineType.PE,)):
                    emit_x_load()
                    emit_s2()
                    emit_s1()

    nc.compile()
    return nc


def _get_nc():
    if "nc" not in _CACHE:
        _CACHE["nc"] = _build_bass()
    return _CACHE["nc"]


def _f8(a):
    return np.clip(np.asarray(a, np.float32), -240, 240).astype(FP8)


def _route(x, gate_W, gate_b):
    """float64 gating: (idxs [N,2], gates [N,2]) matching softmax-top2."""
    logits = x.astype(np.float64) @ gate_W.astype(np.float64) + gate_b.astype(np.float64)
    part = np.argpartition(-logits, TOP_K - 1, axis=1)[:, :TOP_K]
    part_vals = np.take_along_axis(logits, part, axis=1)
    order = np.lexsort((part, -part_vals), axis=1)
    idxs = np.take_along_axis(part, order, axis=1)
    m = logits.max(axis=1, keepdims=True)
    ex = np.exp(logits - m)
    probs = ex / ex.sum(axis=1, keepdims=True)
    gates = np.take_along_axis(probs, idxs, axis=1)
    return idxs, gates


def _make_in_maps(x, W1, b1, W2, idxs, gates):
    if MODE == "C":
        return _make_in_maps_c(x, W1, b1, W2, idxs, gates)
    return _make_in_maps_scheme(x, W1, b1, W2, idxs, gates)


def _make_in_maps_c(x, W1, b1, W2, idxs, gates):
    BF16 = ml_dtypes.bfloat16
    x64 = x.astype(np.float64)
    in_maps, rows_per_e, over_per_e = [], [], []
    for e in range(E):
        rows = np.where((idxs[:, 0] == e) | (idxs[:, 1] == e))[0]
        if len(rows) > C:
            g = np.where(idxs[rows, 0] == e, gates[rows, 0], gates[rows, 1])
            sel = np.argsort(-g, kind="stable")
            keep = np.sort(rows[sel[:C]])
            over = np.sort(rows[sel[C:]])
        else:
            keep, over = rows, rows[:0]
        rows_per_e.append(keep)
        over_per_e.append(over)

        xe = np.zeros((C, D))
        xe[: len(keep)] = x64[keep]
        xs = xe.T * SX
        xa = _f8(xs)
        xb = _f8(2 * xs - xa.astype(np.float64))
        xpair = np.stack([xa.reshape(DT, 128, C), xb.reshape(DT, 128, C)],
                         axis=2).transpose(1, 0, 2, 3)   # [128, DT, 2, C]

        ws = W1[e].astype(np.float64) * SW1
        wa = _f8(ws)
        wb = _f8(2 * ws - wa.astype(np.float64))
        w1pair = np.stack([wa.reshape(DT, 128, H), wb.reshape(DT, 128, H)],
                          axis=2).transpose(1, 0, 2, 3)  # [128, DT, 2, H]

        w2r = W2[e].reshape(HT, 128, D).transpose(1, 0, 2).reshape(128, HT * D)
        b1r = b1[e].reshape(HT, 128).T

        in_maps.append({
            "xt": np.ascontiguousarray(xpair.reshape(128, DT * 2 * C)),
            "w1": np.ascontiguousarray(w1pair.reshape(128, DT * 2 * H)),
            "w2": np.ascontiguousarray(w2r.astype(BF16)),
            "b1t": np.ascontiguousarray(b1r.astype(np.float32)),
        })
    return in_maps, rows_per_e, over_per_e


def _make_in_maps_scheme(x, W1, b1, W2, idxs, gates):
    """Per-core device input dicts + (device rows, overflow rows) per expert."""
    x64 = x.astype(np.float64)
    in_maps, rows_per_e, over_per_e = [], [], []
    for e in range(E):
        rows = np.where((idxs[:, 0] == e) | (idxs[:, 1] == e))[0]
        if len(rows) > C:
            # capacity overflow: keep the C highest-gate tokens on device,
            # compute the rest exactly on host in the combine step.
            g = np.where(idxs[rows, 0] == e, gates[rows, 0], gates[rows, 1])
            sel = np.argsort(-g, kind="stable")
            keep = np.sort(rows[sel[:C]])
            over = np.sort(rows[sel[C:]])
        else:
            keep, over = rows, rows[:0]
        rows_per_e.append(keep)
        over_per_e.append(over)

        xe = np.zeros((C, D))
        xe[: len(keep)] = x64[keep]
        xs = xe.T * SX                                   # [D, C]
        xH = _f8(xs)
        xL = _f8(xs - xH.astype(np.float64))
        xpair = np.stack([xH.reshape(DT, 128, C), xL.reshape(DT, 128, C)],
                         axis=0).transpose(2, 0, 1, 3)   # [128, 2, DT, C]

        ws = W1[e].astype(np.float64) * (2 * SW1)        # [D, H]
        wH = _f8(ws)
        wL = _f8(ws - wH.astype(np.float64))
        w1pair = np.stack([wH.reshape(DT, 128, H), wL.reshape(DT, 128, H)],
                          axis=0).transpose(2, 0, 1, 3)  # [128, 2, DT, H]

        ws2 = W2[e].astype(np.float64) * SW2             # [H, D]
        w2a = _f8(ws2)
        w2b = _f8((2 * ws2 - w2a.astype(np.float64)) / DSHIFT)
        w2pair = np.stack([w2a, w2b], axis=1).reshape(HT, 128, 2, D) \
                   .transpose(1, 0, 2, 3)                # [128, HT, 2, D]

        b1p = np.stack([b1[e] * SH, b1[e] * SH * DSHIFT], axis=1)   # [H, 2]
        b1r = b1p.reshape(HT, 128, 2).transpose(1, 0, 2).reshape(128, 2 * HT)

        in_maps.append({
            "xt": np.ascontiguousarray(xpair.reshape(128, 2 * DT * C)),
            "w1": np.ascontiguousarray(w1pair.reshape(128, 2 * DT * H)),
            "w2": np.ascontiguousarray(w2pair.reshape(128, HT * 2 * D)),
            "b1t": np.ascontiguousarray(b1r.astype(np.float32)),
        })
    return in_maps, rows_per_e, over_per_e


def _get_runner():
    """Compiled SPMD executor (shard_map over 8 cores), cached across calls."""
    if "runner" in _CACHE:
        return _CACHE["runner"]
    import jax
    from jax.sharding import Mesh, PartitionSpec
    from jax.experimental.shard_map import shard_map
    from concourse import mybir
    from concourse.bass2jax import (
        _bass_exec_p, install_neuronx_cc_hook, partition_id_tensor,
    )

    nc = _get_nc()
    install_neuronx_cc_hook()
    partition_name = nc.partition_id_tensor.name if nc.partition_id_tensor else None

    in_names, out_names, out_avals, zero_outs = [], [], [], []
    for alloc in nc.m.functions[0].allocations:
        if not isinstance(alloc, mybir.MemoryLocationSet):
            continue
        name = alloc.memorylocations[0].name
        if alloc.kind == "ExternalInput":
            if name != partition_name:
                in_names.append(name)
        elif alloc.kind == "ExternalOutput":
            out_names.append(name)
            shape, dtype = tuple(alloc.tensor_shape), mybir.dt.np(alloc.dtype)
            out_avals.append(jax.core.ShapedArray(shape, dtype))
            zero_outs.append(np.zeros(shape, dtype))
    n_params = len(in_names)
    all_names = list(in_names) + out_names
    if partition_name is not None:
        all_names.append(partition_name)

    def _body(*args):
        operands = list(args)
        if partition_name is not None:
            operands.append(partition_id_tensor())
        outs = _bass_exec_p.bind(
            *operands, out_avals=tuple(out_avals), in_names=tuple(all_names),
            out_names=tuple(out_names), lowering_input_output_aliases=(),
            sim_require_finite=True, sim_require_nnan=True, nc=nc)
        return tuple(outs)

    devices = jax.devices()[:E]
    mesh = Mesh(np.asarray(devices), ("core",))
    spec = PartitionSpec("core")
    fn = jax.jit(shard_map(
        _body, mesh=mesh,
        in_specs=(spec,) * (n_params + len(out_names)),
        out_specs=(spec,) * len(out_names), check_rep=False))

    def run(in_maps):
        concat = [np.concatenate([np.asarray(m[n]) for m in in_maps], axis=0)
                  for n in in_names]
        concat += [np.concatenate([z] * E, axis=0) for z in zero_outs]
        outs = fn(*concat)
        return [
            {name: np.asarray(outs[i]).reshape(E, *out_avals[i].shape)[c]
             for i, name in enumerate(out_names)}
            for c in range(E)
        ]

    _CACHE["runner"] = run
    return run


def kernel(x, gate_W, gate_b, W1, b1, W2, b2):
    x = np.asarray(x, dtype=np.float32)
    gate_W = np.asarray(gate_W, dtype=np.float32)
    gate_b = np.asarray(gate_b, dtype=np.float32)
    W1 = np.asarray(W1, dtype=np.float32)
    b1 = np.asarray(b1, dtype=np.float32)
    W2 = np.asarray(W2, dtype=np.float32)
    b2 = np.asarray(b2, dtype=np.float32)

    idxs, gates = _route(x, gate_W, gate_b)
    in_maps, rows_per_e, over_per_e = _make_in_maps(x, W1, b1, W2, idxs, gates)

    results = _get_runner()(in_maps)

    out = np.zeros((N, D), dtype=np.float64)
    yscale = 1.0 if MODE == "C" else 1.0 / (2 * SH * SW2)
    x64 = x.astype(np.float64)
    for e in range(E):
        rows = rows_per_e[e]
        y = results[e]["yt"].T[: len(rows)].astype(np.float64) * yscale \
            + b2[e].astype(np.float64)
        g = np.where(idxs[rows, 0] == e, gates[rows, 0], gates[rows, 1])
        out[rows] += g[:, None] * y
        over = over_per_e[e]
        if len(over):
            # exact host path for over-capacity tokens
            hover = np.maximum(
                x64[over] @ W1[e].astype(np.float64) + b1[e].astype(np.float64), 0.0)
            yover = hover @ W2[e].astype(np.float64) + b2[e].astype(np.float64)
            gover = np.where(idxs[over, 0] == e, gates[over, 0], gates[over, 1])
            out[over] += gover[:, None] * yover
    return out.astype(np.float32)
